# revision 30
# baseline (speedup 1.0000x reference)
"""GCN classifier (2x GCNConv + mean-pool + 2-layer MLP) on 8 Trainium2 cores.

Sharding strategy (graph/data parallel per the hint):
- Nodes partitioned contiguously: core c owns dst nodes [c*6250, (c+1)*6250).
- conv1 (aggregate-then-transform): edges + self-loops partitioned by dst
  owner, grouped into 98 windows of 64 dst nodes, padded to 128-edge chunks
  (chunk counts maxed across cores -> one SPMD program). The host ships each
  core its incident edges' x rows pre-scaled by the full sym-norm
  dinv[src]*dinv[dst] and quantized to fp8-e4m3 (chunk-ordered -> one big
  sequential DMA stream per 512-node group). The scatter-add is realized on
  the PE as matmuls with the fp8 x chunk stationary and a 64-wide 0/1
  one-hot (bf16 iota-compare on DVE) as the moving operand, accumulating in
  PSUM -> the aggregation lands feature-major, no transposes. Dense W1
  (bf16) + bias + relu -> h1 kept feature-major in SBUF only.
- conv2 + mean-pool fused algebraically: with no nonlinearity between
  conv2's aggregation and the pooling, pooled sums satisfy
  pool[G] = sum_s A[s,G] * (h1[s] @ W2), where
  A[s,G] = dinv[s]*(sum_{e:src=s,dst in G} dinv[dst] + [batch[s]==G]*dinv[s])
  is built on host from edge_index/batch/deg only (structural data). Each
  core computes p = h1 @ W2 (bf16) for its own node chunks and immediately
  accumulates A_chunk^T @ p_chunk into a persistent [64,256] PSUM tile --
  no halo exchange, no gathers; p never leaves SBUF.
- One 32KB bf16 AllReduce of the pooled partials; mean+bias+relu and the
  tiny MLP run replicated; core 0's output wins.
- Pipelining: x_edges DMA + one-hot build per 512-node group (double
  buffered), aggregation/dense/p-chunks interleaved group by group so the
  PE stays busy (and the HAM clock stays warm) end to end.
"""

import sys
import types

import ml_dtypes
import numpy as np

try:
    import antenv  # noqa: F401

    if "antenv.axon_hooks" not in sys.modules:
        _m = types.ModuleType("antenv.axon_hooks")
        _m._hook = None
        _m.set_axon_ntff_profile_hook = lambda h: setattr(_m, "_hook", h)
        _m.get_axon_ntff_profile_hook = lambda: _m._hook
        sys.modules["antenv.axon_hooks"] = _m
except Exception:
    pass

import concourse.bacc as bacc
import concourse.mybir as mybir
import concourse.tile as tile
from concourse import bass_utils

F32 = mybir.dt.float32
BF16 = mybir.dt.bfloat16
F8 = mybir.dt.float8e4
AF = mybir.ActivationFunctionType
OP = mybir.AluOpType

N = 50000
E = 500000
DIN = 256
DH = 512
NG = 64
DOUT = 16

NCORES = 8
SLICE = N // NCORES  # 6250
WW = 64  # dst window width (one-hot width)
NW = (SLICE + WW - 1) // WW  # 98 windows
NPAD = 6272  # 49 * 128 node columns
NCHK = NPAD // 128  # 49 node chunks
NGRP = 13  # 12 groups of 512 node cols + 1 of 128

_COMPILED: dict = {}


def _group_info(g):
    """(first window, #windows, node col0, #node cols, first chunk, #chunks)"""
    if g < 12:
        return (8 * g, 8, 512 * g, 512, 4 * g, 4)
    return (96, 2, 6144, 128, 48, 1)


def _layout(K1):
    """Batches of <=4 windows: [(g, ws, {w: [(gcol, grel)]}, nch, c0)]."""
    batches = []
    gcol = 0
    for g in range(NGRP):
        w0, nwin, _, _, _, _ = _group_info(g)
        nhalf = 2 if nwin == 8 else 1
        for half in range(nhalf):
            ws = list(range(w0 + half * 4, min(w0 + (half + 1) * 4, w0 + nwin)))
            c0 = gcol
            rel = 0
            wch = {}
            for w in ws:
                lst = []
                for _ in range(int(K1[w])):
                    lst.append((gcol, rel))
                    gcol += 1
                    rel += 1
                wch[w] = lst
            batches.append((g, ws, wch, rel, c0))
    return batches, gcol


def _preprocess(x, edge_index, batch):
    src = np.asarray(edge_index[0], dtype=np.int64)
    dst = np.asarray(edge_index[1], dtype=np.int64)
    batch = np.asarray(batch, dtype=np.int64)

    deg = np.bincount(dst, minlength=N).astype(np.float64) + 1.0
    dinv = (1.0 / np.sqrt(deg)).astype(np.float32)
    cnt = np.maximum(np.bincount(batch, minlength=NG), 1)

    loops = np.arange(N, dtype=np.int64)

    # ---------- conv1: edges + self-loops grouped by (core, 64-window) ----------
    s1 = np.concatenate([src, loops])
    d1 = np.concatenate([dst, loops])
    norm1 = dinv[s1] * dinv[d1]
    core1 = d1 // SLICE
    win1 = (d1 % SLICE) // WW
    key1 = core1 * NW + win1
    order1 = np.argsort(key1, kind="stable")
    ss1, ds1, nn1 = s1[order1], d1[order1], norm1[order1]
    counts1 = np.bincount(key1, minlength=NCORES * NW).reshape(NCORES, NW)
    starts1 = np.zeros(NCORES * NW + 1, dtype=np.int64)
    np.cumsum(counts1.reshape(-1), out=starts1[1:])
    K1 = np.ceil(counts1.max(axis=0) / 128).astype(np.int64)  # [NW]

    meta = tuple(int(v) for v in K1)
    batches, C1 = _layout(K1)

    # ---------- fused conv2+pool coefficient matrix A[s, G] ----------
    gd = batch[dst]
    A = np.bincount(src * NG + gd, weights=dinv[dst].astype(np.float64),
                    minlength=N * NG).reshape(N, NG).astype(np.float32)
    A[loops, batch] += dinv
    A *= dinv[:, None]

    xf = np.asarray(x, np.float32)

    per_core = []
    for c in range(NCORES):
        src_cols = np.zeros((C1, 128), dtype=np.int64)
        nrm_cols = np.zeros((C1, 128), dtype=np.float32)
        dst_cols = np.full((C1, 128), -1.0, dtype=np.float32)
        for _g, ws, wch, _nch, _c0 in batches:
            for w in ws:
                gi = c * NW + w
                e0, e1 = starts1[gi], starts1[gi + 1]
                n_e = int(e1 - e0)
                cols = wch[w]
                k = len(cols)
                sv = np.zeros(k * 128, dtype=np.int64)
                sv[:n_e] = ss1[e0:e1]
                nv = np.zeros(k * 128, dtype=np.float32)
                nv[:n_e] = nn1[e0:e1]
                dv = np.full(k * 128, -1.0, dtype=np.float32)
                dv[:n_e] = (ds1[e0:e1] - (c * SLICE + w * WW)).astype(np.float32)
                for j, (gcol, _r) in enumerate(cols):
                    src_cols[gcol] = sv[j * 128 : (j + 1) * 128]
                    nrm_cols[gcol] = nv[j * 128 : (j + 1) * 128]
                    dst_cols[gcol] = dv[j * 128 : (j + 1) * 128]
        rows = xf[src_cols.reshape(-1)] * nrm_cols.reshape(-1)[:, None]
        x_edges = np.ascontiguousarray(
            rows.astype(ml_dtypes.float8_e4m3).reshape(C1, 128, DIN).transpose(1, 0, 2)
        ).reshape(128, C1 * DIN)

        Ac = np.zeros((NPAD, NG), dtype=np.float32)
        Ac[:SLICE] = A[c * SLICE : (c + 1) * SLICE]
        a_sb = np.ascontiguousarray(
            Ac.reshape(NCHK, 128, NG).transpose(1, 0, 2)
        ).reshape(128, NCHK * NG).astype(ml_dtypes.bfloat16)

        per_core.append(
            dict(
                x_edges=x_edges,
                dst1=np.ascontiguousarray(dst_cols.T).astype(ml_dtypes.bfloat16),
                a_mat=a_sb,
            )
        )
    return meta, per_core, cnt.astype(np.float32)


def _build_program(meta):
    K1 = np.array(meta)
    batches, C1 = _layout(K1)

    nc = bacc.Bacc("TRN2", target_bir_lowering=False, debug=False, num_devices=NCORES)

    def din(name, shape, dt=F32):
        return nc.dram_tensor(name, shape, dt, kind="ExternalInput").ap()

    x_edges = din("x_edges", [128, C1 * DIN], F8)
    dst1 = din("dst1", [128, C1], BF16)
    a_mat = din("a_mat", [128, NCHK * NG], BF16)
    iota64 = din("iota64", [128, WW], BF16)
    w1b = din("w1b", [128, 2 * DH], BF16)
    w2b = din("w2b", [128, 4 * (DH // 2)], BF16)
    b1c = din("b1c", [128, DH // 128])
    b2r = din("b2r", [128, DH // 2])
    wf1 = din("wf1", [128, 2 * (DH // 4)])
    bf1c = din("bf1c", [128, 1])
    wf2 = din("wf2", [DH // 4, DOUT])
    bf2c = din("bf2c", [DOUT, 1])
    cnt_in = din("cnt", [NG, 1])
    ident = din("ident", [128, 128])
    out = nc.dram_tensor("out", [NG, DOUT], F32, kind="ExternalOutput").ap()

    with tile.TileContext(nc) as tc:
        with (
            tc.tile_pool(name="const", bufs=1) as cp,
            tc.tile_pool(name="big", bufs=1) as bigp,
            tc.tile_pool(name="work", bufs=1) as wp,
            tc.tile_pool(name="psum", bufs=1, space="PSUM") as pp,
            tc.tile_pool(name="dram", bufs=1, space="DRAM") as dp,
        ):
            def load(ap_in, shape, dt=F32, pool=cp):
                t = pool.tile(shape, dt, name=ap_in.tensor.name + "_sb")
                nc.scalar.dma_start(t[:], ap_in[:])
                return t

            # loads gating the pipeline start go first; the rest after batch 0
            dst1_sb = load(dst1, [128, C1], BF16)
            iota_sb = load(iota64, [128, WW], BF16)

            h1s = [bigp.tile([128, NPAD], BF16, name=f"h1s_{k}") for k in range(4)]

            sfm_groups: dict = {}

            def sfm_of(g):
                if g not in sfm_groups:
                    sfm_groups[g] = [
                        wp.tile([128, 512], BF16, tag=f"sfm{h}", bufs=2, name=f"sfm{h}_{g}")
                        for h in range(2)
                    ]
                return sfm_groups[g]

            pgx = pp.tile([NG, DH // 2], F32, name="pgx")
            g_local = dp.tile([NG, DH // 2], BF16)
            g_red = dp.tile([NG, DH // 2], BF16, addr_space="Shared")

            def emit_stream(batchpair):
                """One G1 DMA + one-hot build covering a whole group's batches."""
                c0 = batchpair[0][4]
                nch = sum(b[3] for b in batchpair)
                g = batchpair[0][0]
                G1 = wp.tile([128, nch, DIN], F8, tag="G1", bufs=3, name=f"g1g_{g}")
                nc.sync.dma_start(
                    G1[:].rearrange("p c d -> p (c d)"),
                    x_edges[:, c0 * DIN : (c0 + nch) * DIN],
                )
                oh = wp.tile([128, nch, WW], BF16, tag="oh", bufs=3, name=f"ohg_{g}")
                nc.vector.tensor_tensor(
                    out=oh[:],
                    in0=iota_sb[:].rearrange("p (o i) -> p o i", o=1).to_broadcast([128, nch, WW]),
                    in1=dst1_sb[:, c0 : c0 + nch].rearrange("p (c o) -> p c o", o=1).to_broadcast([128, nch, WW]),
                    op=OP.is_equal,
                )
                return G1, oh, c0

            def emit_batch(g, ws, wch, nch, c0, G1, oh, gc0):
                rel0 = c0 - gc0
                sf = sfm_of(g)
                nw = len(ws)
                wb0 = ws[0] - 8 * g
                pa = pp.tile([128, nw, 2, WW], F32, tag="agg", bufs=2, name=f"pa_{ws[0]}")
                for w in ws:
                    cols = wch[w]
                    wrel = w - ws[0]
                    for j, (_gcol, grel) in enumerate(cols):
                        for h in range(2):
                            nc.tensor.matmul(
                                out=pa[:, wrel, h, :],
                                lhsT=G1[:, rel0 + grel, h * 128 : (h + 1) * 128],
                                rhs=oh[:, rel0 + grel, :],
                                start=(j == 0),
                                stop=(j == len(cols) - 1),
                            )
                for h in range(2):
                    nc.scalar.activation(
                        sf[h][:, wb0 * WW : (wb0 + nw) * WW],
                        pa[:, :, h, :],
                        AF.Copy,
                    )

            def emit_dense(g):
                _, _, n0, ncols, _, _ = _group_info(g)
                sf = sfm_of(g)
                for m in range(4):
                    ph = pp.tile([128, 512], F32, tag="h1", bufs=2, name=f"ph_{g}_{m}")
                    for k in range(2):
                        nc.tensor.matmul(
                            out=ph[:, :ncols],
                            lhsT=w1_sb[:, k * DH + m * 128 : k * DH + (m + 1) * 128],
                            rhs=sf[k][:, :ncols],
                            start=(k == 0),
                            stop=(k == 1),
                        )
                    nc.scalar.activation(
                        h1s[m][:, n0 : n0 + ncols], ph[:, :ncols], AF.Relu,
                        bias=b1_sb[:, m : m + 1],
                    )

            def emit_pA(cc):
                c0 = cc * 128
                ppm = pp.tile([128, DH // 2], F32, tag="p2", bufs=2, name=f"ppm_{cc}")
                for k in range(4):
                    nc.tensor.matmul(
                        out=ppm[:],
                        lhsT=h1s[k][:, c0 : c0 + 128],
                        rhs=w2_sb[:, k * (DH // 2) : (k + 1) * (DH // 2)],
                        start=(k == 0),
                        stop=(k == 3),
                    )
                pb = wp.tile([128, DH // 2], BF16, tag="pb", bufs=2, name=f"pb_{cc}")
                nc.vector.tensor_copy(pb[:], ppm[:])
                nc.tensor.matmul(
                    out=pgx[:],
                    lhsT=a_sb[:, cc * NG : (cc + 1) * NG],
                    rhs=pb[:],
                    start=(cc == 0),
                    stop=(cc == NCHK - 1),
                )

            bidx = 0
            streams = {0: emit_stream(batches[0:2]), 1: emit_stream(batches[2:4])}
            w1_sb = load(w1b, [128, 2 * DH], BF16)
            b1_sb = load(b1c, [128, DH // 128])
            for g in range(NGRP):
                _, nwin, _, _, cc0, nccs = _group_info(g)
                nb = 2 if nwin == 8 else 1
                bp = batches[bidx : bidx + nb]
                G1, oh, gc0 = streams.pop(g) if g in streams else emit_stream(bp)
                for b in bp:
                    emit_batch(*b, G1, oh, gc0)
                    bidx += 1
                if g == 0:
                    a_sb = load(a_mat, [128, NCHK * NG], BF16)
                    w2_sb = load(w2b, [128, 4 * (DH // 2)], BF16)
                    b2_sb = load(b2r, [128, DH // 2])
                    wf1_sb = load(wf1, [128, 2 * (DH // 4)])
                    bf1_sb = load(bf1c, [128, 1])
                    wf2_sb = load(wf2, [DH // 4, DOUT])
                    bf2_sb = load(bf2c, [DOUT, 1])
                    cnt_sb = load(cnt_in, [NG, 1])
                    idf32 = load(ident, [128, 128])
                emit_dense(g)
                for cc in range(cc0, cc0 + nccs):
                    emit_pA(cc)

            # ---------------- tail: AllReduce of the pooled partial ----------------
            gsb = wp.tile([NG, DH // 2], BF16, name="gsb")
            nc.vector.tensor_copy(gsb[:], pgx[:])
            nc.sync.dma_start(g_local[:], gsb[:])
            nc.gpsimd.collective_compute(
                "AllReduce",
                OP.add,
                replica_groups=[list(range(NCORES))],
                ins=[g_local.opt()],
                outs=[g_red.opt()],
            )
            cur = wp.tile([NG, DH // 2], BF16, name="gsum")
            nc.sync.dma_start(cur[:], g_red[:])

            cinv = wp.tile([NG, 1], F32)
            nc.vector.reciprocal(cinv[:], cnt_sb[:])
            gmean = wp.tile([NG, DH // 2], F32)
            nc.vector.scalar_tensor_tensor(
                out=gmean[:],
                in0=cur[:],
                scalar=cinv[:, 0:1],
                in1=b2_sb[:NG, :],
                op0=OP.mult,
                op1=OP.add,
            )
            grelu = wp.tile([NG, DH // 2], F32)
            nc.scalar.activation(grelu[:], gmean[:], AF.Relu)

            g_fm = [wp.tile([128, NG], F32, name=f"gfm_{k}") for k in range(2)]
            for k in range(2):
                pt = pp.tile([128, NG], F32, tag="t", bufs=1, name=f"gt_{k}")
                nc.tensor.transpose(pt[:], grelu[:, k * 128 : (k + 1) * 128], idf32[:NG, :NG])
                nc.vector.tensor_copy(g_fm[k][:], pt[:])
            pz = pp.tile([128, NG], F32, tag="h1", bufs=2, name="pz")
            for k in range(2):
                nc.tensor.matmul(
                    out=pz[:],
                    lhsT=wf1_sb[:, k * 128 : (k + 1) * 128],
                    rhs=g_fm[k][:],
                    start=(k == 0),
                    stop=(k == 1),
                )
            zsb = wp.tile([128, NG], F32)
            nc.scalar.activation(zsb[:], pz[:], AF.Relu, bias=bf1_sb[:, 0:1])
            po = pp.tile([DOUT, NG], F32, tag="t", bufs=1, name="po")
            nc.tensor.matmul(out=po[:], lhsT=wf2_sb[:], rhs=zsb[:], start=True, stop=True)
            osb = wp.tile([DOUT, NG], F32)
            nc.scalar.activation(osb[:], po[:], AF.Relu, bias=bf2_sb[:, 0:1])
            pout = pp.tile([NG, DOUT], F32, tag="t", bufs=1, name="pout")
            nc.tensor.transpose(pout[:], osb[:], idf32[:DOUT, :DOUT])
            out_sb = wp.tile([NG, DOUT], F32)
            nc.vector.tensor_copy(out_sb[:], pout[:])
            nc.sync.dma_start(out[:], out_sb[:])

    nc.compile()
    return nc


def _get_program(meta):
    if meta not in _COMPILED:
        _COMPILED[meta] = _build_program(meta)
    return _COMPILED[meta]


def _make_in_maps(W1, b1, W2, b2, Wf1, bf1, Wf2, bf2, per_core, cnt):
    bf = ml_dtypes.bfloat16
    W1 = np.asarray(W1, np.float32)
    W2 = np.asarray(W2, np.float32)
    Wf1 = np.asarray(Wf1, np.float32)
    shared = dict(
        iota64=np.tile(np.arange(WW, dtype=np.float32)[None, :], (128, 1)).astype(bf),
        w1b=np.ascontiguousarray(
            np.concatenate([W1[0:128, :], W1[128:256, :]], axis=1)
        ).astype(bf),
        w2b=np.ascontiguousarray(
            np.concatenate([W2[k * 128 : (k + 1) * 128, :] for k in range(4)], axis=1)
        ).astype(bf),
        b1c=np.ascontiguousarray(np.asarray(b1, np.float32).reshape(DH // 128, 128).T),
        b2r=np.ascontiguousarray(np.tile(np.asarray(b2, np.float32)[None, :], (128, 1))),
        wf1=np.ascontiguousarray(np.concatenate([Wf1[0:128, :], Wf1[128:256, :]], axis=1)),
        bf1c=np.tile(np.asarray(bf1, np.float32).reshape(DH // 4, 1), (1, 1)),
        wf2=np.asarray(Wf2, np.float32),
        bf2c=np.asarray(bf2, np.float32).reshape(DOUT, 1),
        cnt=np.asarray(cnt, np.float32).reshape(NG, 1),
        ident=np.eye(128, dtype=np.float32),
    )
    return [dict(shared, **per_core[c]) for c in range(NCORES)]


def kernel(
    x, W1, b1, W2, b2, Wf1, bf1, Wf2, bf2, edge_index, batch, num_graphs, _trace=False
):
    assert int(num_graphs) == NG
    meta, per_core, cnt = _preprocess(
        np.asarray(x), np.asarray(edge_index), np.asarray(batch)
    )
    nc = _get_program(meta)
    in_maps = _make_in_maps(W1, b1, W2, b2, Wf1, bf1, Wf2, bf2, per_core, cnt)
    res = bass_utils.run_bass_kernel_spmd(
        nc, in_maps, core_ids=list(range(NCORES)), trace=_trace
    )
    out = np.asarray(res.results[0]["out"], np.float32)
    if _trace:
        kernel._last_results = res
    return out


# revision 31
# speedup vs baseline: 1.0604x; 1.0604x over previous
"""GCN classifier (2x GCNConv + mean-pool + 2-layer MLP) on 8 Trainium2 cores.

Sharding strategy (graph/data parallel per the hint):
- Nodes partitioned contiguously: core c owns dst nodes [c*6250, (c+1)*6250).
- conv1 (aggregate-then-transform): edges + self-loops partitioned by dst
  owner, grouped into 98 windows of 64 dst nodes, padded to 128-edge chunks
  (chunk counts maxed across cores -> one SPMD program). The host ships each
  core its incident edges' x rows pre-scaled by the full sym-norm
  dinv[src]*dinv[dst] and quantized to fp8-e4m3 (chunk-ordered -> one big
  sequential DMA stream per 512-node group). The scatter-add is realized on
  the PE as matmuls with the fp8 x chunk stationary and a 64-wide 0/1
  one-hot (bf16 iota-compare on DVE) as the moving operand, accumulating in
  PSUM -> the aggregation lands feature-major, no transposes. Dense W1
  (bf16) + bias + relu -> h1 kept feature-major in SBUF only.
- conv2 + mean-pool fused algebraically: with no nonlinearity between
  conv2's aggregation and the pooling, pooled sums satisfy
  pool[G] = sum_s A[s,G] * (h1[s] @ W2), where
  A[s,G] = dinv[s]*(sum_{e:src=s,dst in G} dinv[dst] + [batch[s]==G]*dinv[s])
  is built on host from edge_index/batch/deg only (structural data). Each
  core computes p = h1 @ W2 (bf16) for its own node chunks and immediately
  accumulates A_chunk^T @ p_chunk into a persistent [64,256] PSUM tile --
  no halo exchange, no gathers; p never leaves SBUF.
- One 32KB bf16 AllReduce of the pooled partials; mean+bias+relu and the
  tiny MLP run replicated; core 0's output wins.
- Pipelining: x_edges DMA + one-hot build per 512-node group (double
  buffered), aggregation/dense/p-chunks interleaved group by group so the
  PE stays busy (and the HAM clock stays warm) end to end.
"""

import sys
import types

import ml_dtypes
import numpy as np

try:
    import antenv  # noqa: F401

    if "antenv.axon_hooks" not in sys.modules:
        _m = types.ModuleType("antenv.axon_hooks")
        _m._hook = None
        _m.set_axon_ntff_profile_hook = lambda h: setattr(_m, "_hook", h)
        _m.get_axon_ntff_profile_hook = lambda: _m._hook
        sys.modules["antenv.axon_hooks"] = _m
except Exception:
    pass

import concourse.bacc as bacc
import concourse.mybir as mybir
import concourse.tile as tile
from concourse import bass_utils

F32 = mybir.dt.float32
BF16 = mybir.dt.bfloat16
F8 = mybir.dt.float8e4
AF = mybir.ActivationFunctionType
OP = mybir.AluOpType

N = 50000
E = 500000
DIN = 256
DH = 512
NG = 64
DOUT = 16

NCORES = 8
SLICE = N // NCORES  # 6250
WW = 64  # dst window width (one-hot width)
NW = (SLICE + WW - 1) // WW  # 98 windows
NPAD = 6272  # 49 * 128 node columns
NCHK = NPAD // 128  # 49 node chunks
NGRP = 13  # 12 groups of 512 node cols + 1 of 128

_COMPILED: dict = {}


def _group_info(g):
    """(first window, #windows, node col0, #node cols, first chunk, #chunks)"""
    if g < 12:
        return (8 * g, 8, 512 * g, 512, 4 * g, 4)
    return (96, 2, 6144, 128, 48, 1)


def _layout(K1):
    """Batches of <=4 windows: [(g, ws, {w: [(gcol, grel)]}, nch, c0)]."""
    batches = []
    gcol = 0
    for g in range(NGRP):
        w0, nwin, _, _, _, _ = _group_info(g)
        nhalf = 2 if nwin == 8 else 1
        for half in range(nhalf):
            ws = list(range(w0 + half * 4, min(w0 + (half + 1) * 4, w0 + nwin)))
            c0 = gcol
            rel = 0
            wch = {}
            for w in ws:
                lst = []
                for _ in range(int(K1[w])):
                    lst.append((gcol, rel))
                    gcol += 1
                    rel += 1
                wch[w] = lst
            batches.append((g, ws, wch, rel, c0))
    return batches, gcol


def _preprocess(x, edge_index, batch):
    src = np.asarray(edge_index[0], dtype=np.int64)
    dst = np.asarray(edge_index[1], dtype=np.int64)
    batch = np.asarray(batch, dtype=np.int64)

    deg = np.bincount(dst, minlength=N).astype(np.float64) + 1.0
    dinv = (1.0 / np.sqrt(deg)).astype(np.float32)
    cnt = np.maximum(np.bincount(batch, minlength=NG), 1)

    loops = np.arange(N, dtype=np.int64)

    # ---------- conv1: edges + self-loops grouped by (core, 64-window) ----------
    s1 = np.concatenate([src, loops])
    d1 = np.concatenate([dst, loops])
    norm1 = dinv[s1] * dinv[d1]
    core1 = d1 // SLICE
    win1 = (d1 % SLICE) // WW
    key1 = core1 * NW + win1
    order1 = np.argsort(key1, kind="stable")
    ss1, ds1, nn1 = s1[order1], d1[order1], norm1[order1]
    counts1 = np.bincount(key1, minlength=NCORES * NW).reshape(NCORES, NW)
    starts1 = np.zeros(NCORES * NW + 1, dtype=np.int64)
    np.cumsum(counts1.reshape(-1), out=starts1[1:])
    K1 = np.ceil(counts1.max(axis=0) / 128).astype(np.int64)  # [NW]

    meta = tuple(int(v) for v in K1)
    batches, C1 = _layout(K1)

    # ---------- fused conv2+pool coefficient matrix A[s, G] ----------
    gd = batch[dst]
    A = np.bincount(src * NG + gd, weights=dinv[dst].astype(np.float64),
                    minlength=N * NG).reshape(N, NG).astype(np.float32)
    A[loops, batch] += dinv
    A *= dinv[:, None]

    xf = np.asarray(x, np.float32)

    per_core = []
    for c in range(NCORES):
        src_cols = np.zeros((C1, 128), dtype=np.int64)
        nrm_cols = np.zeros((C1, 128), dtype=np.float32)
        dst_cols = np.full((C1, 128), -1.0, dtype=np.float32)
        for _g, ws, wch, _nch, _c0 in batches:
            for w in ws:
                gi = c * NW + w
                e0, e1 = starts1[gi], starts1[gi + 1]
                n_e = int(e1 - e0)
                cols = wch[w]
                k = len(cols)
                sv = np.zeros(k * 128, dtype=np.int64)
                sv[:n_e] = ss1[e0:e1]
                nv = np.zeros(k * 128, dtype=np.float32)
                nv[:n_e] = nn1[e0:e1]
                dv = np.full(k * 128, -1.0, dtype=np.float32)
                dv[:n_e] = (ds1[e0:e1] - (c * SLICE + w * WW)).astype(np.float32)
                for j, (gcol, _r) in enumerate(cols):
                    src_cols[gcol] = sv[j * 128 : (j + 1) * 128]
                    nrm_cols[gcol] = nv[j * 128 : (j + 1) * 128]
                    dst_cols[gcol] = dv[j * 128 : (j + 1) * 128]
        rows = xf[src_cols.reshape(-1)] * nrm_cols.reshape(-1)[:, None]
        x_edges = np.ascontiguousarray(
            rows.astype(ml_dtypes.float8_e4m3).reshape(C1, 128, DIN).transpose(1, 0, 2)
        ).reshape(128, C1 * DIN)

        Ac = np.zeros((NPAD, NG), dtype=np.float32)
        Ac[:SLICE] = A[c * SLICE : (c + 1) * SLICE]
        a_sb = np.ascontiguousarray(
            Ac.reshape(NCHK, 128, NG).transpose(1, 0, 2)
        ).reshape(128, NCHK * NG).astype(ml_dtypes.bfloat16)

        per_core.append(
            dict(
                x_edges=x_edges,
                dst1=np.ascontiguousarray(dst_cols.T).astype(ml_dtypes.bfloat16),
                a_mat=a_sb,
            )
        )
    return meta, per_core, cnt.astype(np.float32)


def _build_program(meta):
    K1 = np.array(meta)
    batches, C1 = _layout(K1)

    nc = bacc.Bacc("TRN2", target_bir_lowering=False, debug=False, num_devices=NCORES)

    def din(name, shape, dt=F32):
        return nc.dram_tensor(name, shape, dt, kind="ExternalInput").ap()

    x_edges = din("x_edges", [128, C1 * DIN], F8)
    dst1 = din("dst1", [128, C1], BF16)
    a_mat = din("a_mat", [128, NCHK * NG], BF16)
    iota64 = din("iota64", [128, WW], BF16)
    w1b = din("w1b", [128, 2 * DH], BF16)
    w2b = din("w2b", [128, 4 * (DH // 2)], BF16)
    b1c = din("b1c", [128, DH // 128])
    b2r = din("b2r", [128, DH // 2])
    wf1 = din("wf1", [128, 2 * (DH // 4)])
    bf1c = din("bf1c", [128, 1])
    wf2 = din("wf2", [DH // 4, DOUT])
    bf2c = din("bf2c", [DOUT, 1])
    cnt_in = din("cnt", [NG, 1])
    ident = din("ident", [128, 128])
    out = nc.dram_tensor("out", [NG, DOUT], F32, kind="ExternalOutput").ap()

    with tile.TileContext(nc) as tc:
        with (
            tc.tile_pool(name="const", bufs=1) as cp,
            tc.tile_pool(name="big", bufs=1) as bigp,
            tc.tile_pool(name="work", bufs=1) as wp,
            tc.tile_pool(name="psum", bufs=1, space="PSUM") as pp,
            tc.tile_pool(name="dram", bufs=1, space="DRAM") as dp,
        ):
            def load(ap_in, shape, dt=F32, pool=cp):
                t = pool.tile(shape, dt, name=ap_in.tensor.name + "_sb")
                nc.sync.dma_start(t[:], ap_in[:])
                return t

            # loads gating the pipeline start go first; the rest after batch 0
            dst1_sb = load(dst1, [128, C1], BF16)
            iota_sb = load(iota64, [128, WW], BF16)

            h1s = [bigp.tile([128, NPAD], BF16, name=f"h1s_{k}") for k in range(4)]

            sfm_groups: dict = {}

            def sfm_of(g):
                if g not in sfm_groups:
                    sfm_groups[g] = [
                        wp.tile([128, 512], BF16, tag=f"sfm{h}", bufs=2, name=f"sfm{h}_{g}")
                        for h in range(2)
                    ]
                return sfm_groups[g]

            pgx = pp.tile([NG, DH // 2], F32, name="pgx")
            g_local = dp.tile([NG, DH // 2], BF16)
            g_red = dp.tile([NG, DH // 2], BF16, addr_space="Shared")

            def emit_stream(batchpair):
                """One G1 DMA + one-hot build covering a whole group's batches."""
                c0 = batchpair[0][4]
                nch = sum(b[3] for b in batchpair)
                g = batchpair[0][0]
                G1 = wp.tile([128, nch, DIN], F8, tag="G1", bufs=3, name=f"g1g_{g}")
                nc.sync.dma_start(
                    G1[:].rearrange("p c d -> p (c d)"),
                    x_edges[:, c0 * DIN : (c0 + nch) * DIN],
                )
                oh = wp.tile([128, nch, WW], BF16, tag="oh", bufs=3, name=f"ohg_{g}")
                nc.vector.tensor_tensor(
                    out=oh[:],
                    in0=iota_sb[:].rearrange("p (o i) -> p o i", o=1).to_broadcast([128, nch, WW]),
                    in1=dst1_sb[:, c0 : c0 + nch].rearrange("p (c o) -> p c o", o=1).to_broadcast([128, nch, WW]),
                    op=OP.is_equal,
                )
                return G1, oh, c0

            def emit_batch(g, ws, wch, nch, c0, G1, oh, gc0):
                rel0 = c0 - gc0
                sf = sfm_of(g)
                nw = len(ws)
                wb0 = ws[0] - 8 * g
                pa = pp.tile([128, nw, 2, WW], F32, tag="agg", bufs=2, name=f"pa_{ws[0]}")
                for w in ws:
                    cols = wch[w]
                    wrel = w - ws[0]
                    for j, (_gcol, grel) in enumerate(cols):
                        for h in range(2):
                            nc.tensor.matmul(
                                out=pa[:, wrel, h, :],
                                lhsT=G1[:, rel0 + grel, h * 128 : (h + 1) * 128],
                                rhs=oh[:, rel0 + grel, :],
                                start=(j == 0),
                                stop=(j == len(cols) - 1),
                            )
                for h in range(2):
                    nc.scalar.activation(
                        sf[h][:, wb0 * WW : (wb0 + nw) * WW],
                        pa[:, :, h, :],
                        AF.Copy,
                    )

            def emit_dense(g):
                _, _, n0, ncols, _, _ = _group_info(g)
                sf = sfm_of(g)
                for m in range(4):
                    ph = pp.tile([128, 512], F32, tag="h1", bufs=2, name=f"ph_{g}_{m}")
                    for k in range(2):
                        nc.tensor.matmul(
                            out=ph[:, :ncols],
                            lhsT=w1_sb[:, k * DH + m * 128 : k * DH + (m + 1) * 128],
                            rhs=sf[k][:, :ncols],
                            start=(k == 0),
                            stop=(k == 1),
                        )
                    nc.scalar.activation(
                        h1s[m][:, n0 : n0 + ncols], ph[:, :ncols], AF.Relu,
                        bias=b1_sb[:, m : m + 1],
                    )

            def emit_pA(cc):
                c0 = cc * 128
                ppm = pp.tile([128, DH // 2], F32, tag="p2", bufs=2, name=f"ppm_{cc}")
                for k in range(4):
                    nc.tensor.matmul(
                        out=ppm[:],
                        lhsT=h1s[k][:, c0 : c0 + 128],
                        rhs=w2_sb[:, k * (DH // 2) : (k + 1) * (DH // 2)],
                        start=(k == 0),
                        stop=(k == 3),
                    )
                pb = wp.tile([128, DH // 2], BF16, tag="pb", bufs=2, name=f"pb_{cc}")
                nc.vector.tensor_copy(pb[:], ppm[:])
                nc.tensor.matmul(
                    out=pgx[:],
                    lhsT=a_sb[:, cc * NG : (cc + 1) * NG],
                    rhs=pb[:],
                    start=(cc == 0),
                    stop=(cc == NCHK - 1),
                )

            bidx = 0
            streams = {0: emit_stream(batches[0:2]), 1: emit_stream(batches[2:4])}
            w1_sb = load(w1b, [128, 2 * DH], BF16)
            b1_sb = load(b1c, [128, DH // 128])
            for g in range(NGRP):
                _, nwin, _, _, cc0, nccs = _group_info(g)
                nb = 2 if nwin == 8 else 1
                bp = batches[bidx : bidx + nb]
                G1, oh, gc0 = streams.pop(g) if g in streams else emit_stream(bp)
                for b in bp:
                    emit_batch(*b, G1, oh, gc0)
                    bidx += 1
                if g == 0:
                    a_sb = load(a_mat, [128, NCHK * NG], BF16)
                    w2_sb = load(w2b, [128, 4 * (DH // 2)], BF16)
                    b2_sb = load(b2r, [128, DH // 2])
                    wf1_sb = load(wf1, [128, 2 * (DH // 4)])
                    bf1_sb = load(bf1c, [128, 1])
                    wf2_sb = load(wf2, [DH // 4, DOUT])
                    bf2_sb = load(bf2c, [DOUT, 1])
                    cnt_sb = load(cnt_in, [NG, 1])
                    idf32 = load(ident, [128, 128])
                emit_dense(g)
                for cc in range(cc0, cc0 + nccs):
                    emit_pA(cc)

            # ---------------- tail: AllReduce of the pooled partial ----------------
            gsb = wp.tile([NG, DH // 2], BF16, name="gsb")
            nc.vector.tensor_copy(gsb[:], pgx[:])
            nc.sync.dma_start(g_local[:], gsb[:])
            nc.gpsimd.collective_compute(
                "AllReduce",
                OP.add,
                replica_groups=[list(range(NCORES))],
                ins=[g_local.opt()],
                outs=[g_red.opt()],
            )
            cur = wp.tile([NG, DH // 2], BF16, name="gsum")
            nc.sync.dma_start(cur[:], g_red[:])

            cinv = wp.tile([NG, 1], F32)
            nc.vector.reciprocal(cinv[:], cnt_sb[:])
            gmean = wp.tile([NG, DH // 2], F32)
            nc.vector.scalar_tensor_tensor(
                out=gmean[:],
                in0=cur[:],
                scalar=cinv[:, 0:1],
                in1=b2_sb[:NG, :],
                op0=OP.mult,
                op1=OP.add,
            )
            grelu = wp.tile([NG, DH // 2], F32)
            nc.scalar.activation(grelu[:], gmean[:], AF.Relu)

            g_fm = [wp.tile([128, NG], F32, name=f"gfm_{k}") for k in range(2)]
            for k in range(2):
                pt = pp.tile([128, NG], F32, tag="t", bufs=1, name=f"gt_{k}")
                nc.tensor.transpose(pt[:], grelu[:, k * 128 : (k + 1) * 128], idf32[:NG, :NG])
                nc.vector.tensor_copy(g_fm[k][:], pt[:])
            pz = pp.tile([128, NG], F32, tag="h1", bufs=2, name="pz")
            for k in range(2):
                nc.tensor.matmul(
                    out=pz[:],
                    lhsT=wf1_sb[:, k * 128 : (k + 1) * 128],
                    rhs=g_fm[k][:],
                    start=(k == 0),
                    stop=(k == 1),
                )
            zsb = wp.tile([128, NG], F32)
            nc.scalar.activation(zsb[:], pz[:], AF.Relu, bias=bf1_sb[:, 0:1])
            po = pp.tile([DOUT, NG], F32, tag="t", bufs=1, name="po")
            nc.tensor.matmul(out=po[:], lhsT=wf2_sb[:], rhs=zsb[:], start=True, stop=True)
            osb = wp.tile([DOUT, NG], F32)
            nc.scalar.activation(osb[:], po[:], AF.Relu, bias=bf2_sb[:, 0:1])
            pout = pp.tile([NG, DOUT], F32, tag="t", bufs=1, name="pout")
            nc.tensor.transpose(pout[:], osb[:], idf32[:DOUT, :DOUT])
            out_sb = wp.tile([NG, DOUT], F32)
            nc.vector.tensor_copy(out_sb[:], pout[:])
            nc.sync.dma_start(out[:], out_sb[:])

    nc.compile()
    return nc


def _get_program(meta):
    if meta not in _COMPILED:
        _COMPILED[meta] = _build_program(meta)
    return _COMPILED[meta]


def _make_in_maps(W1, b1, W2, b2, Wf1, bf1, Wf2, bf2, per_core, cnt):
    bf = ml_dtypes.bfloat16
    W1 = np.asarray(W1, np.float32)
    W2 = np.asarray(W2, np.float32)
    Wf1 = np.asarray(Wf1, np.float32)
    shared = dict(
        iota64=np.tile(np.arange(WW, dtype=np.float32)[None, :], (128, 1)).astype(bf),
        w1b=np.ascontiguousarray(
            np.concatenate([W1[0:128, :], W1[128:256, :]], axis=1)
        ).astype(bf),
        w2b=np.ascontiguousarray(
            np.concatenate([W2[k * 128 : (k + 1) * 128, :] for k in range(4)], axis=1)
        ).astype(bf),
        b1c=np.ascontiguousarray(np.asarray(b1, np.float32).reshape(DH // 128, 128).T),
        b2r=np.ascontiguousarray(np.tile(np.asarray(b2, np.float32)[None, :], (128, 1))),
        wf1=np.ascontiguousarray(np.concatenate([Wf1[0:128, :], Wf1[128:256, :]], axis=1)),
        bf1c=np.tile(np.asarray(bf1, np.float32).reshape(DH // 4, 1), (1, 1)),
        wf2=np.asarray(Wf2, np.float32),
        bf2c=np.asarray(bf2, np.float32).reshape(DOUT, 1),
        cnt=np.asarray(cnt, np.float32).reshape(NG, 1),
        ident=np.eye(128, dtype=np.float32),
    )
    return [dict(shared, **per_core[c]) for c in range(NCORES)]


def kernel(
    x, W1, b1, W2, b2, Wf1, bf1, Wf2, bf2, edge_index, batch, num_graphs, _trace=False
):
    assert int(num_graphs) == NG
    meta, per_core, cnt = _preprocess(
        np.asarray(x), np.asarray(edge_index), np.asarray(batch)
    )
    nc = _get_program(meta)
    in_maps = _make_in_maps(W1, b1, W2, b2, Wf1, bf1, Wf2, bf2, per_core, cnt)
    res = bass_utils.run_bass_kernel_spmd(
        nc, in_maps, core_ids=list(range(NCORES)), trace=_trace
    )
    out = np.asarray(res.results[0]["out"], np.float32)
    if _trace:
        kernel._last_results = res
    return out
